# revision 18
# baseline (speedup 1.0000x reference)
"""CenterLossLayer Trainium2 kernel (8-core SPMD).

Reference computation (B=4096 samples, C=100000 classes, D=128):
    gathered      = centers[labels]                      # via dense one-hot matmul
    delta[c]      = cnt_c * centers[c] - sum_{i: l_i=c} x_i
    new_centers   = centers - 0.5 * delta / (cnt + 1)
    result_i      = ||x_i - gathered_i||^2

Sharding: batch split 8 ways (512 samples/core) for the compute;
classes split 8 ways (12500 rows/core) for the new_centers output.
Each core:
  1. bulk-copies its centers class-shard -> output shard (dominant memory traffic)
  2. computes per-sample patches p_i = (sx_i - cnt_i*g_i) * 0.5/(cnt_i*(cnt_i+1))
     for its 512 samples, where cnt/sx are segment counts/sums over the FULL
     batch obtained with an equality-matrix matmul on the tensor engine
  3. AllGathers the new-row values (every core then holds all 4096 rows)
  4. scatter-writes all 4096 rows into its own output shard via indirect DMA
     (indices clamped to a trash row for classes outside the shard).
     v_i = g - 0.5*(cnt*g - sx)/(cnt+1) is the FINAL row value; duplicate
     labels produce bitwise-identical v on every core, so colliding writes
     are benign.  (dma_scatter_add was tried first but its hardware ucode
     reads the RMW base from row 0 of the destination instead of row idx.)
"""

import os
import sys

import numpy as np

for _p in ("/opt/trn_rl_repo", "/root/.axon_site/_ro/trn_rl_repo"):
    if os.path.isdir(_p) and _p not in sys.path:
        sys.path.insert(0, _p)

import concourse.bass as bass
import concourse.bacc as bacc
import concourse.mybir as mybir
import concourse.tile as tile
from concourse import bass_utils

B, C, D = 4096, 100000, 128
NCORES = 8
BS = B // NCORES          # 512 samples per core
CS = C // NCORES          # 12500 classes per core
TRASH = CS                # extra row that absorbs out-of-shard scatters
NB = B // 128             # 32 all-batch chunks
NM = BS // 128            # 4 own-batch chunks

F32 = mybir.dt.float32
I32 = mybir.dt.int32
I16 = mybir.dt.int16

_NC_CACHE = None


def _build_nc(dbg=False):
    nc = bacc.Bacc(
        "TRN2", target_bir_lowering=False, debug=False, num_devices=NCORES
    )
    if dbg:
        gm_dump = nc.dram_tensor("gm_dump", [BS, D], F32, kind="ExternalOutput")
        cnt_dump = nc.dram_tensor("cnt_dump", [128, NM], F32, kind="ExternalOutput")
        vown_dump = nc.dram_tensor("vown_dump", [BS, D], F32, kind="ExternalOutput")
        vall_dump = nc.dram_tensor("vall_dump", [B, D], F32, kind="ExternalOutput")

    x_own = nc.dram_tensor("x_own", [BS, D], F32, kind="ExternalInput")
    x_all = nc.dram_tensor("x_all", [B, D], F32, kind="ExternalInput")
    lab_own = nc.dram_tensor("lab_own", [1, BS], F32, kind="ExternalInput")
    lab_all_pt = nc.dram_tensor("lab_all_pt", [128, NB], F32, kind="ExternalInput")
    gidx_pt = nc.dram_tensor("gidx_pt", [128, NM], I32, kind="ExternalInput")
    valid_pt = nc.dram_tensor("valid_pt", [128, NM], F32, kind="ExternalInput")
    soff = nc.dram_tensor("soff", [128, NB], I32, kind="ExternalInput")
    centers_all = nc.dram_tensor("centers_all", [C, D], F32, kind="ExternalInput")
    centers_shard = nc.dram_tensor("centers_shard", [CS, D], F32, kind="ExternalInput")

    result_own = nc.dram_tensor("result_own", [BS, 1], F32, kind="ExternalOutput")
    out_shard = nc.dram_tensor("new_centers_shard", [CS + 1, D], F32, kind="ExternalOutput")

    v_own_d = nc.dram_tensor("v_own_d", [BS, D], F32)
    v_all_d = nc.dram_tensor("v_all_d", [B, D], F32, addr_space="Shared")

    with tile.TileContext(nc) as tc:
        with (
            tc.tile_pool(name="sbuf", bufs=1) as cpool,
            tc.tile_pool(name="ebuf", bufs=3) as epool,
            tc.tile_pool(name="psum", bufs=1, space="PSUM") as ppool,
        ):
            # 1) bulk copy centers shard -> output shard (DRAM->DRAM)
            n_pieces = 4
            rows = CS // n_pieces
            for i in range(n_pieces):
                nc.sync.dma_start(
                    out=out_shard[i * rows : (i + 1) * rows, :],
                    in_=centers_shard[i * rows : (i + 1) * rows, :],
                )

            # 2) stage inputs in SBUF
            lab_all_sb = cpool.tile([128, NB], F32)
            nc.sync.dma_start(out=lab_all_sb[:], in_=lab_all_pt[:, :])
            lab_own_sb = cpool.tile([1, BS], F32)
            nc.sync.dma_start(out=lab_own_sb[:], in_=lab_own[:, :])
            gidx_sb = cpool.tile([128, NM], I32)
            nc.sync.dma_start(out=gidx_sb[:], in_=gidx_pt[:, :])
            valid_sb = cpool.tile([128, NM], F32)
            nc.sync.dma_start(out=valid_sb[:], in_=valid_pt[:, :])
            soff_sb = cpool.tile([128, NB], I32)
            nc.sync.dma_start(out=soff_sb[:], in_=soff[:, :])

            # memset whole tile to 1.0 first, then overwrite cols 0..D-1 with x:
            # the D-th column of every block stays 1.0 and yields counts in the
            # segment matmul.  (A strided memset of just that column works in
            # CoreSim but writes the wrong cells on hardware.)
            xe = cpool.tile([128, NB, D + 1], F32)
            nc.vector.memset(xe[:, :, :], 1.0)
            nc.sync.dma_start(
                out=xe[:, :, 0:D], in_=x_all.rearrange("(t p) d -> p t d", p=128)
            )

            x_own_sb = cpool.tile([128, NM, D], F32)
            nc.sync.dma_start(
                out=x_own_sb[:], in_=x_own.rearrange("(t p) d -> p t d", p=128)
            )

            # broadcast own labels across partitions: ones^T @ lab_own
            ones1 = cpool.tile([1, 128], F32)
            nc.vector.memset(ones1[:], 1.0)
            bc_psum = ppool.tile([128, BS], F32)
            nc.tensor.matmul(
                bc_psum[:], lhsT=ones1[:], rhs=lab_own_sb[:], start=True, stop=True
            )
            bcast_sb = cpool.tile([128, BS], F32)
            nc.vector.tensor_copy(bcast_sb[:], bc_psum[:])

            # gather own centers rows g = centers[labels_own]
            g_sb = cpool.tile([128, NM, D], F32)
            for m in range(NM):
                nc.gpsimd.indirect_dma_start(
                    out=g_sb[:, m, :],
                    out_offset=None,
                    in_=centers_all[:, :],
                    in_offset=bass.IndirectOffsetOnAxis(ap=gidx_sb[:, m : m + 1], axis=0),
                )

            # 3) equality matrix chunks + segment-sum matmuls
            # psum[m][s, :] accumulates [sum_x (D cols) | count (last col)]
            ps = []
            for m in range(NM):
                ps.append(
                    ppool.tile([128, D + 1], F32, tag=f"ps{m}", name=f"ps{m}")
                )
            for j in range(NB):
                e_t = epool.tile([128, BS], F32, tag="e")
                nc.vector.tensor_scalar(
                    out=e_t[:],
                    in0=bcast_sb[:],
                    scalar1=lab_all_sb[:, j : j + 1],
                    scalar2=None,
                    op0=mybir.AluOpType.is_equal,
                )
                for m in range(NM):
                    nc.tensor.matmul(
                        ps[m][:],
                        lhsT=e_t[:, m * 128 : (m + 1) * 128],
                        rhs=xe[:, j, :],
                        start=(j == 0),
                        stop=(j == NB - 1),
                    )

            # 4) per-chunk epilogue: result + patches
            v_sb = cpool.tile([128, NM, D], F32)
            for m in range(NM):
                sx = cpool.tile([128, D + 1], F32, tag=f"sx{m}")
                nc.vector.tensor_copy(sx[:], ps[m][:])
                cnt = sx[:, D : D + 1]

                gm = cpool.tile([128, D], F32, tag=f"gm{m}")
                nc.vector.tensor_scalar(
                    out=gm[:],
                    in0=g_sb[:, m, :],
                    scalar1=valid_sb[:, m : m + 1],
                    scalar2=None,
                    op0=mybir.AluOpType.mult,
                )

                if dbg:
                    nc.sync.dma_start(
                        out=gm_dump[m * 128 : (m + 1) * 128, :], in_=gm[:]
                    )
                    nc.sync.dma_start(out=cnt_dump[:, m : m + 1], in_=cnt)

                diff = cpool.tile([128, D], F32, tag=f"diff{m}")
                nc.vector.tensor_tensor(
                    out=diff[:], in0=x_own_sb[:, m, :], in1=gm[:],
                    op=mybir.AluOpType.subtract,
                )
                sq = cpool.tile([128, D], F32, tag=f"sq{m}")
                nc.vector.tensor_tensor(
                    out=sq[:], in0=diff[:], in1=diff[:], op=mybir.AluOpType.mult
                )
                res = cpool.tile([128, 1], F32, tag=f"res{m}")
                nc.vector.tensor_reduce(
                    out=res[:], in_=sq[:], axis=mybir.AxisListType.X,
                    op=mybir.AluOpType.add,
                )
                nc.sync.dma_start(
                    out=result_own[m * 128 : (m + 1) * 128, :], in_=res[:]
                )

                # v = g + 0.5*(sx - cnt*g)/(cnt+1)   (the FINAL new-center row)
                den = cpool.tile([128, 1], F32, tag=f"den{m}")
                nc.vector.tensor_scalar(
                    out=den[:], in0=cnt, scalar1=1.0, scalar2=None,
                    op0=mybir.AluOpType.add,
                )
                rec = cpool.tile([128, 1], F32, tag=f"rec{m}")
                nc.vector.reciprocal(rec[:], den[:])
                rec2 = cpool.tile([128, 1], F32, tag=f"rec2{m}")
                nc.vector.tensor_scalar(
                    out=rec2[:], in0=rec[:], scalar1=0.5, scalar2=None,
                    op0=mybir.AluOpType.mult,
                )
                t1 = cpool.tile([128, D], F32, tag=f"t1{m}")
                nc.vector.tensor_scalar(
                    out=t1[:], in0=gm[:], scalar1=cnt, scalar2=None,
                    op0=mybir.AluOpType.mult,
                )
                t2 = cpool.tile([128, D], F32, tag=f"t2{m}")
                nc.vector.tensor_tensor(
                    out=t2[:], in0=sx[:, 0:D], in1=t1[:],
                    op=mybir.AluOpType.subtract,
                )
                t3 = cpool.tile([128, D], F32, tag=f"t3{m}")
                nc.vector.tensor_scalar(
                    out=t3[:], in0=t2[:], scalar1=rec2, scalar2=None,
                    op0=mybir.AluOpType.mult,
                )
                nc.vector.tensor_tensor(
                    out=v_sb[:, m, :], in0=gm[:], in1=t3[:],
                    op=mybir.AluOpType.add,
                )

            # 5) exchange patches: every core ends up with all B rows
            if dbg:
                nc.sync.dma_start(
                    out=vown_dump.rearrange("(t p) d -> p t d", p=128), in_=v_sb[:]
                )
            nc.sync.dma_start(
                out=v_own_d.rearrange("(t p) d -> p t d", p=128), in_=v_sb[:]
            )
            nc.gpsimd.collective_compute(
                "AllGather",
                mybir.AluOpType.bypass,
                replica_groups=[list(range(NCORES))],
                ins=[v_own_d[:, :].opt()],
                outs=[v_all_d[:, :].opt()],
            )

            # 6) scatter-add all patches into own output shard
            v_all_sb = cpool.tile([128, NB, D], F32)
            nc.sync.dma_start(
                out=v_all_sb[:], in_=v_all_d.rearrange("(t p) d -> p t d", p=128)
            )
            if dbg:
                nc.sync.dma_start(
                    out=vall_dump.rearrange("(t p) d -> p t d", p=128),
                    in_=v_all_sb[:],
                )
            for t in range(NB):
                nc.gpsimd.indirect_dma_start(
                    out=out_shard[:, :],
                    out_offset=bass.IndirectOffsetOnAxis(
                        ap=soff_sb[:, t : t + 1], axis=0
                    ),
                    in_=v_all_sb[:, t, :],
                    in_offset=None,
                )

    nc.compile()
    return nc


def _get_nc():
    global _NC_CACHE
    if _NC_CACHE is None:
        _NC_CACHE = _build_nc()
    return _NC_CACHE


def _make_in_maps(x, onehot, centers):
    x = np.ascontiguousarray(np.asarray(x, dtype=np.float32))
    centers = np.ascontiguousarray(np.asarray(centers, dtype=np.float32))
    onehot = np.asarray(onehot)

    labels = np.argmax(onehot, axis=1).astype(np.int64)
    valid = np.asarray(onehot[np.arange(B), labels]) > 0.5
    labf = np.where(valid, labels.astype(np.float32), np.float32(-1.0)).astype(
        np.float32
    )
    gidx = np.where(valid, labels, 0).astype(np.int32)
    validf = valid.astype(np.float32)

    lab_all_pt = np.ascontiguousarray(labf.reshape(NB, 128).T)

    in_maps = []
    for k in range(NCORES):
        sl = slice(k * BS, (k + 1) * BS)
        loc = labels - k * CS
        ok = valid & (loc >= 0) & (loc < CS)
        loc32 = np.where(ok, loc, TRASH).astype(np.int32)
        in_maps.append(
            {
                "x_own": x[sl],
                "x_all": x,
                "lab_own": np.ascontiguousarray(labf[sl].reshape(1, BS)),
                "lab_all_pt": lab_all_pt,
                "gidx_pt": np.ascontiguousarray(gidx[sl].reshape(NM, 128).T),
                "valid_pt": np.ascontiguousarray(validf[sl].reshape(NM, 128).T),
                "soff": np.ascontiguousarray(loc32.reshape(NB, 128).T),
                "centers_all": centers,
                "centers_shard": centers[k * CS : (k + 1) * CS],
            }
        )
    return in_maps


def _assemble(results):
    result = np.concatenate(
        [results[k]["result_own"] for k in range(NCORES)], axis=0
    ).astype(np.float32)
    new_centers = np.concatenate(
        [results[k]["new_centers_shard"][:CS] for k in range(NCORES)], axis=0
    ).astype(np.float32)
    return result, new_centers


def run_traced(x, onehot, centers, trace=True):
    """Run on hardware with NTFF profiling; returns ((result, new_centers), exec_ns)."""
    nc = _get_nc()
    in_maps = _make_in_maps(x, onehot, centers)
    res = bass_utils.run_bass_kernel_spmd(
        nc, in_maps, list(range(NCORES)), trace=trace
    )
    return _assemble(res.results), res.exec_time_ns


def kernel(x, onehot, centers):
    nc = _get_nc()
    in_maps = _make_in_maps(x, onehot, centers)
    res = bass_utils.run_bass_kernel_spmd(nc, in_maps, list(range(NCORES)))
    return _assemble(res.results)
